# revision 1
# baseline (speedup 1.0000x reference)
"""Multi-head attention (16 heads, head_dim 64, B=2, S=2048) on 8 trn2 cores.

Sharding: tensor-parallel over heads — core i computes heads 2i, 2i+1 for both
batch elements. Each core receives the full X^T [1024, 4096] plus its 128-row
slice of Wq/Wk/Wv (transposed to [1024, 128] lhsT layout), and returns
ctx^T [128, 4096] (2 heads x 64 dims) x (2 batches x 2048 positions),
already softmax-normalized. Host assembles the [2, 2048, 1024] output.

Device algorithm (all matmuls in float32r: full PE speed, ~1e-3 max rel err):
  QT/KT/VT [128, 4096] = W^T.T @ X^T  (accumulate 8 k-tiles of 128)
  RoPE: q' = q * cos + (P.T @ q) * sin_signed, P = rotate-half permutation
  V transposed to token-major [128, 32, 2*65] with a ones column per head
  attention in 8 groups (b, h, q-half of 1024), scores psum double-buffered:
    per 128-token k-chunk: scoresT [128k, 1024q] = KT_chunk.T @ QT
    E = exp(scoresT / 8)  (ScalarE, PSUM -> SBUF, one instr per k-chunk)
    ctx_psum [65, 1024q] += Vaug_chunk.T @ E  (row 64 = softmax denominators)
  normalize: denominators reshaped [1,1024]->[128,8] via DMA so the
  reciprocal uses all lanes, partition-broadcast on GpSimd, DVE multiply.
Phase-1 chunks and attention groups are emitted interleaved so the Tile
scheduler can overlap them (batch-0 groups only need token chunks 0-3).
"""
import os
import sys

for _p in ("/opt/trn_rl_repo", "/root/.axon_site/_ro/trn_rl_repo"):
    if os.path.isdir(_p) and _p not in sys.path:
        sys.path.insert(0, _p)

import numpy as np

import concourse.bass as bass  # noqa: F401
import concourse.mybir as mybir
import concourse.tile as tile
from concourse import bacc
from concourse.bass_utils import run_bass_kernel_spmd

dt = mybir.dt

B, S, NH, HD = 2, 2048, 16, 64
H = NH * HD            # 1024
T = B * S              # 4096
NCORES = 8
HPC = NH // NCORES     # heads per core = 2
DPC = HPC * HD         # dims per core = 128
CHUNK = 512            # token chunk for QKV projection
NCHUNK = T // CHUNK    # 8
KC = 128               # k-token chunk in attention
NKC = S // KC          # 16 per batch
QG = 1024              # q extent per attention group
NQN = QG // 512        # 2 matmul subtiles per group row
VW = HD + 1            # 65: 64 dims then the ones column (softmax denom)
KTILES = H // 128      # 8

_prog_cache = {}
_last_in_maps = None


def _build_program():
    nc = bacc.Bacc("TRN2", target_bir_lowering=False, debug=False,
                   num_devices=NCORES)
    f32 = dt.float32
    f32r = dt.float32r

    xt_d = nc.declare_dram_parameter("xt", [NCHUNK, 128, KTILES, CHUNK], f32r,
                                     isOutput=False)
    wq_d = nc.declare_dram_parameter("wq", [128, KTILES, DPC], f32r,
                                     isOutput=False)
    wk_d = nc.declare_dram_parameter("wk", [128, KTILES, DPC], f32r,
                                     isOutput=False)
    wv_d = nc.declare_dram_parameter("wv", [128, KTILES, DPC], f32r,
                                     isOutput=False)
    cos_d = nc.declare_dram_parameter("cos2", [DPC, S], f32, isOutput=False)
    sin_d = nc.declare_dram_parameter("sins", [DPC, S], f32, isOutput=False)
    perm_d = nc.declare_dram_parameter("perm", [DPC, DPC], f32r, isOutput=False)
    ident_d = nc.declare_dram_parameter("ident", [128, 128], f32, isOutput=False)
    ctxt_d = nc.declare_dram_parameter("ctxt", [DPC, T], f32, isOutput=True)

    Exp = mybir.ActivationFunctionType.Exp

    with tile.TileContext(nc) as tc:
        with (
            tc.tile_pool(name="persist", bufs=1) as pp,
            tc.tile_pool(name="consts", bufs=1) as cp,
            tc.tile_pool(name="p1", bufs=2) as p1,
            tc.tile_pool(name="p1s", bufs=3) as p1s,
            tc.tile_pool(name="p2e", bufs=3) as p2e,
            tc.tile_pool(name="p2o", bufs=2) as p2o,
            # PSUM: scores 2x2 banks + phase1 2x1 + ctx 2 = 8 banks
            tc.tile_pool(name="psA", bufs=2, space="PSUM") as psA,
            tc.tile_pool(name="psP", bufs=2, space="PSUM") as psP,
            tc.tile_pool(name="psCX", bufs=1, space="PSUM") as psCX,
        ):
            qt = pp.tile([DPC, T], f32r, tag="qt")
            kt = pp.tile([DPC, T], f32r, tag="kt")
            vaug = pp.tile([128, T // 128, HPC * VW], f32r, tag="vaug")

            cos_sb = cp.tile([DPC, S], f32, tag="cos")
            sin_sb = cp.tile([DPC, S], f32, tag="sin")
            perm_sb = cp.tile([DPC, DPC], f32r, tag="perm")
            ident = cp.tile([128, 128], f32, tag="ident")
            wq_sb = cp.tile([128, KTILES, DPC], f32r, tag="wq")
            wk_sb = cp.tile([128, KTILES, DPC], f32r, tag="wk")
            wv_sb = cp.tile([128, KTILES, DPC], f32r, tag="wv")

            warm = cp.tile([128, 128], f32, tag="warm")
            nc.vector.memset(warm[:], 0.5)
            wps = psP.tile([128, 128], f32, tag="P")
            for _ in range(14):
                nc.tensor.matmul(wps[:], warm[:], warm[:],
                                 start=True, stop=True)
            xt_tiles = {}

            def load_chunk(c):
                xt_t = p1.tile([128, KTILES, CHUNK], f32r, tag="xt")
                half = KTILES // 2
                nc.sync.dma_start(out=xt_t[:, 0:half, :],
                                  in_=xt_d[c, :, 0:half, :])
                nc.sync.dma_start(out=xt_t[:, half:KTILES, :],
                                  in_=xt_d[c, :, half:KTILES, :])
                xt_tiles[c] = xt_t

            # DMA issue order = first-use order: Wq, chunk 0, Wk/Wv,
            # chunk 1, then RoPE tables and the rest.
            nc.sync.dma_start(out=wq_sb[:], in_=wq_d[:])
            # HAM-keepalive bursts gated on arriving DMAs: keep the PE
            # busy through the input-stream window so chunk 0 runs warm.
            for k in range(6):
                nc.tensor.matmul(wps[:], wq_sb[:, k, :],
                                 wq_sb[:, k + 1, :], start=True, stop=True)
            load_chunk(0)
            for w_sb, w_d in ((wk_sb, wk_d), (wv_sb, wv_d)):
                nc.sync.dma_start(out=w_sb[:], in_=w_d[:])
            for k in range(6):
                nc.tensor.matmul(wps[:], wk_sb[:, k, :],
                                 wk_sb[:, k + 1, :], start=True, stop=True)
            for k in range(6):
                nc.tensor.matmul(wps[:], wv_sb[:, k, :],
                                 wv_sb[:, k + 1, :], start=True, stop=True)
            load_chunk(1)
            nc.sync.dma_start(out=cos_sb[:], in_=cos_d[:])
            nc.sync.dma_start(out=sin_sb[:], in_=sin_d[:])
            nc.sync.dma_start(out=perm_sb[:], in_=perm_d[:])
            nc.sync.dma_start(out=ident[:], in_=ident_d[:])
            ones_sb = cp.tile([128, T // 128], f32, tag="ones")
            nc.vector.memset(ones_sb[:], 1.0)
            for h in range(HPC):
                nc.vector.tensor_copy(
                    vaug[:, :, h * VW + HD:h * VW + HD + 1], ones_sb[:])

            def chunk_pieces(c):
                """QKV projection + RoPE + V transpose for token chunk c,
                as a generator of small emission pieces (PE filler)."""
                if c not in xt_tiles:
                    load_chunk(c)
                xt_t = xt_tiles.pop(c)
                pos = (c * CHUNK) % S
                cs = cos_sb[:, pos:pos + CHUNK]
                sn = sin_sb[:, pos:pos + CHUNK]

                for w_sb, dst in ((wq_sb, qt), (wk_sb, kt)):
                    ps = psP.tile([DPC, CHUNK], f32, tag="P")
                    for k in range(KTILES):
                        nc.tensor.matmul(
                            ps[:], w_sb[:, k, :], xt_t[:, k, :],
                            start=(k == 0), stop=(k == KTILES - 1))
                    raw = p1s.tile([DPC, CHUNK], f32r, tag="raw")
                    nc.scalar.copy(raw[:], ps[:])
                    rot = p1s.tile([DPC, CHUNK], f32, tag="rot")
                    rawf = raw[:].bitcast(f32)
                    hh = HD // 2
                    for blk in range(DPC // hh):
                        sb = ((blk // 2) * 2) + (1 - blk % 2)
                        nc.sync.dma_start(
                            out=rot[blk * hh:(blk + 1) * hh, :],
                            in_=rawf[sb * hh:(sb + 1) * hh, :])
                    t1 = p1s.tile([DPC, CHUNK], f32, tag="t1")
                    nc.vector.tensor_mul(t1[:], rawf, cs)
                    t2 = p1s.tile([DPC, CHUNK], f32, tag="t2")
                    nc.vector.tensor_mul(t2[:], rot[:], sn)
                    nc.vector.tensor_add(
                        dst[:, c * CHUNK:(c + 1) * CHUNK], t1[:], t2[:])
                    yield

                psv = psP.tile([DPC, CHUNK], f32, tag="P")
                for k in range(KTILES):
                    nc.tensor.matmul(
                        psv[:], wv_sb[:, k, :], xt_t[:, k, :],
                        start=(k == 0), stop=(k == KTILES - 1))
                vt = p1s.tile([DPC, CHUNK], f32, tag="vt")
                nc.scalar.copy(vt[:], psv[:])
                for j in range(CHUNK // 128):
                    tp = psP.tile([128, 128], f32, tag="P")
                    nc.tensor.transpose(tp[:], vt[:, j * 128:(j + 1) * 128],
                                        ident[:])
                    tt = c * (CHUNK // 128) + j
                    for h in range(HPC):
                        nc.vector.tensor_copy(
                            vaug[:, tt, h * VW:h * VW + HD],
                            tp[:, h * HD:(h + 1) * HD])
                yield

            def emit_chunk(c):
                for _ in chunk_pieces(c):
                    pass

            def emit_group(b, h, qh, kc_lo=0, kc_hi=NKC, cx=None):
                """Attention for one (batch, head, 1024-wide q half).
                Can be emitted in k-chunk spans; pass cx between spans."""
                hs = slice(h * HD, (h + 1) * HD)
                q0 = b * S + qh * QG
                if cx is None:
                    cx = psCX.tile([VW, QG], f32, tag="cx")
                for kc in range(kc_lo, kc_hi):
                    k0 = b * S + kc * KC
                    sc = psA.tile([128, QG], f32, tag="A")
                    for qn in range(NQN):
                        nc.tensor.matmul(
                            sc[:, qn * 512:(qn + 1) * 512],
                            kt[hs, k0:k0 + KC],
                            qt[hs, q0 + qn * 512:q0 + (qn + 1) * 512],
                            start=True, stop=True)
                    e = p2e.tile([128, QG], f32r, tag="e")
                    nc.scalar.activation(e[:], sc[:], Exp, scale=0.125)
                    tt = (b * S) // 128 + kc
                    for qn in range(NQN):
                        nc.tensor.matmul(
                            cx[:, qn * 512:(qn + 1) * 512],
                            vaug[:, tt, h * VW:(h + 1) * VW],
                            e[:, qn * 512:(qn + 1) * 512],
                            start=(kc == 0), stop=(kc == NKC - 1))
                if kc_hi < NKC:
                    return cx
                # normalize: row HD of cx holds the softmax denominators
                ctxu = p2o.tile([VW, QG], f32, tag="ctxu")
                nc.vector.tensor_copy(ctxu[:], cx[:])
                rsq = p2o.tile([128, QG // 128], f32, tag="rsq")
                nc.sync.dma_start(out=rsq[:], in_=ctxu[HD:VW, :])
                rsqi = p2o.tile([128, QG // 128], f32, tag="rsqi")
                nc.vector.reciprocal(rsqi[:], rsq[:])
                r0 = p2o.tile([1, QG], f32, tag="r0")
                nc.sync.dma_start(out=r0[:], in_=rsqi[:])
                rb = p2o.tile([HD, QG], f32, tag="rb")
                nc.gpsimd.partition_broadcast(rb[:], r0[:])
                oc = p2o.tile([HD, QG], f32, tag="oc")
                nc.vector.tensor_mul(oc[:], ctxu[0:HD, :], rb[:])
                nc.sync.dma_start(
                    out=ctxt_d[h * HD:(h + 1) * HD, q0:q0 + QG], in_=oc[:])

            # Groups for batch b read kt/vaug across ALL of batch b's
            # chunks, so those chunks must be emitted first (Tile tracks
            # deps in emission order).  Batch-1 chunks are emitted AFTER
            # the batch-0 groups: their data deps let them run early, but
            # their low priority makes them pure PE gap-filler during the
            # ACT-bound batch-0 exp stream.
            # The first group of each batch is split: its k-chunks 0-7
            # only need the batch's first two token chunks, starting the
            # exp stream ~20us earlier.
            emit_chunk(0)
            emit_chunk(1)
            cxA = emit_group(0, 0, 0, 0, 8)
            emit_chunk(2)
            emit_chunk(3)
            emit_group(0, 0, 0, 8, NKC, cxA)
            for h, qh in ((1, 0), (0, 1), (1, 1)):
                emit_group(0, h, qh)
            emit_chunk(4)
            emit_chunk(5)
            cxB = emit_group(1, 0, 0, 0, 8)
            emit_chunk(6)
            emit_chunk(7)
            emit_group(1, 0, 0, 8, NKC, cxB)
            for h, qh in ((1, 0), (0, 1), (1, 1)):
                emit_group(1, h, qh)

    nc.compile()
    return nc


def _host_tables():
    inv_freq = 1.0 / (10000.0 ** (np.arange(0, HD, 2, dtype=np.float32) / HD))
    t = np.arange(S, dtype=np.float32)
    freqs = np.outer(t, inv_freq)            # [S, 32]
    emb = np.concatenate([freqs, freqs], axis=-1)  # [S, 64]
    cosT = np.cos(emb).T.astype(np.float32)  # [64, S]
    sinT = np.sin(emb).T.astype(np.float32)
    sin_signed = sinT.copy()
    sin_signed[:HD // 2] *= -1.0             # rows d<32 multiply -sin
    cos2 = np.ascontiguousarray(np.vstack([cosT, cosT]))         # [128, S]
    sins = np.ascontiguousarray(np.vstack([sin_signed, sin_signed]))
    perm = np.zeros((DPC, DPC), dtype=np.float32)
    for d in range(DPC):
        h, j = divmod(d, HD)
        perm[h * HD + (j + HD // 2) % HD, d] = 1.0
    ident = np.eye(128, dtype=np.float32)
    return cos2, sins, perm, ident


def kernel(hidden_states: np.ndarray, Wq: np.ndarray, Wk: np.ndarray,
           Wv: np.ndarray) -> np.ndarray:
    hidden_states = np.asarray(hidden_states, dtype=np.float32)
    Wq = np.asarray(Wq, dtype=np.float32)
    Wk = np.asarray(Wk, dtype=np.float32)
    Wv = np.asarray(Wv, dtype=np.float32)
    assert hidden_states.shape == (B, S, H), hidden_states.shape

    if "nc" not in _prog_cache:
        _prog_cache["nc"] = _build_program()
    nc = _prog_cache["nc"]

    xt = hidden_states.reshape(T, H).T  # [1024, 4096] view
    # pre-tile so each chunk DMA is contiguous per partition:
    # xt_tiled[c, p, k, t] = xt[k*128 + p, c*512 + t]
    xt_tiled = np.ascontiguousarray(
        xt.reshape(KTILES, 128, NCHUNK, CHUNK).transpose(2, 1, 0, 3))
    cos2, sins, perm, ident = _host_tables()

    def tile_w(W):
        # w_tiled[p, k, d] = W.T[k*128 + p, d]
        return np.ascontiguousarray(
            W.T.reshape(KTILES, 128, DPC).transpose(1, 0, 2))

    in_maps = []
    for i in range(NCORES):
        rows = slice(i * DPC, (i + 1) * DPC)
        in_maps.append({
            "xt": xt_tiled,
            "wq": tile_w(Wq[rows]),
            "wk": tile_w(Wk[rows]),
            "wv": tile_w(Wv[rows]),
            "cos2": cos2,
            "sins": sins,
            "perm": perm,
            "ident": ident,
        })

    global _last_in_maps
    _last_in_maps = in_maps
    res = run_bass_kernel_spmd(nc, in_maps, list(range(NCORES)))

    # ctxt per core: [128 (2 heads x 64 dims), 4096 (2 batches x 2048)]
    full = np.stack([res.results[i]["ctxt"] for i in range(NCORES)])
    out = full.reshape(NCORES, HPC, HD, B, S).transpose(3, 4, 0, 1, 2)
    return np.ascontiguousarray(out.reshape(B, S, H), dtype=np.float32)



# revision 3
# speedup vs baseline: 1.0649x; 1.0649x over previous
"""Multi-head attention (16 heads, head_dim 64, B=2, S=2048) on 8 trn2 cores.

Sharding: tensor-parallel over heads — core i computes heads 2i, 2i+1 for both
batch elements. Each core receives the full X^T [1024, 4096] in bf16 plus its
128-row slice of Wq/Wk/Wv (bf16, transposed to [1024, 128] lhsT layout), and
returns ctx^T [128, 4096] f32, already softmax-normalized.

v2 design (exp-stream-bound):
  All matmul operands bf16 (1 cycle/row, cheap LDWEIGHTS, half DMA), psum f32.
  Phase 1: QT/KT [128, 4096] bf16 = W^T.T @ X^T (8 k-tiles), RoPE on DVE
    (rotate-half via SBUF-SBUF DMA partition swap, signed-sin tables).
    V transposed token-major into vaug [128, 32, 2*65] bf16 with a ones
    column per head (softmax denominators fall out of the ctx matmul).
  Attention: one global pipeline over 128 (group, k-chunk) iterations,
    groups strictly sequential: per iteration
      scoresT [128k, 1024q] psum = KT_chunk.T @ QT   (2 matmuls, psA x2)
      e bf16 = exp(scores/8)                          (ACT, the bottleneck)
      ctx[65, 1024] += Vaug_chunk.T @ e               (lagged by 3 iterations)
    The ctx lag + 6 e-buffers keep the ACT stream dense across group
    boundaries while the normalize chain holds the ctx psum.
  Normalize: denom row 64 of ctx -> DVE reciprocal on [1,1024] -> GpSimd
    partition-broadcast -> DVE multiply -> DMA out (f32).
  Phase-1 work for chunks 2-7 is emitted in small pieces (<=4 matmuls)
  woven between attention iterations to fill PE/ACT slack.
"""
import os
import sys

for _p in ("/opt/trn_rl_repo", "/root/.axon_site/_ro/trn_rl_repo"):
    if os.path.isdir(_p) and _p not in sys.path:
        sys.path.insert(0, _p)

import numpy as np
import ml_dtypes

import concourse.bass as bass  # noqa: F401
import concourse.mybir as mybir
import concourse.tile as tile
from concourse import bacc
from concourse.bass_utils import run_bass_kernel_spmd

dt = mybir.dt
BF16 = ml_dtypes.bfloat16

B, S, NH, HD = 2, 2048, 16, 64
H = NH * HD            # 1024
T = B * S              # 4096
NCORES = 8
HPC = NH // NCORES     # heads per core = 2
DPC = HPC * HD         # dims per core = 128
CHUNK = 512            # token chunk for QKV projection
NCHUNK = T // CHUNK    # 8
KC = 128               # k-token chunk in attention
NKC = S // KC          # 16 per batch
QG = 1024              # q extent per attention group
VW = HD + 1            # 65: 64 dims then the ones column (softmax denom)
KTILES = H // 128      # 8
LAG = 3                # ctx matmul lag (iterations) behind scores/exp

_prog_cache = {}
_last_in_maps = None


def _build_program():
    nc = bacc.Bacc("TRN2", target_bir_lowering=False, debug=False,
                   num_devices=NCORES)
    f32 = dt.float32
    bf = dt.bfloat16

    xt_d = nc.declare_dram_parameter("xt", [NCHUNK, 128, KTILES, CHUNK], bf,
                                     isOutput=False)
    wq_d = nc.declare_dram_parameter("wq", [128, KTILES, DPC], bf,
                                     isOutput=False)
    wk_d = nc.declare_dram_parameter("wk", [128, KTILES, DPC], bf,
                                     isOutput=False)
    wv_d = nc.declare_dram_parameter("wv", [128, KTILES, DPC], bf,
                                     isOutput=False)
    cos_d = nc.declare_dram_parameter("cos2", [DPC, S], bf, isOutput=False)
    sin_d = nc.declare_dram_parameter("sins", [DPC, S], bf, isOutput=False)
    ident_d = nc.declare_dram_parameter("ident", [128, 128], f32,
                                        isOutput=False)
    ctxt_d = nc.declare_dram_parameter("ctxt", [DPC, T], f32, isOutput=True)

    Exp = mybir.ActivationFunctionType.Exp

    with tile.TileContext(nc) as tc:
        with (
            tc.tile_pool(name="persist", bufs=1) as pp,
            tc.tile_pool(name="consts", bufs=1) as cp,
            tc.tile_pool(name="p1", bufs=3) as p1,
            tc.tile_pool(name="p1s", bufs=3) as p1s,
            tc.tile_pool(name="p2e", bufs=6) as p2e,
            tc.tile_pool(name="p2o", bufs=2) as p2o,
            # PSUM: scores 2x2 banks + phase1/transpose 2x1 + ctx 1x2 = 8
            tc.tile_pool(name="psA", bufs=2, space="PSUM") as psA,
            tc.tile_pool(name="psP", bufs=2, space="PSUM") as psP,
            tc.tile_pool(name="psCX", bufs=1, space="PSUM") as psCX,
        ):
            qt = pp.tile([DPC, T], bf, tag="qt")
            kt = pp.tile([DPC, T], bf, tag="kt")
            vaug = pp.tile([128, T // 128, HPC * VW], bf, tag="vaug")

            cos_sb = cp.tile([DPC, S], bf, tag="cos")
            sin_sb = cp.tile([DPC, S], bf, tag="sin")
            ident = cp.tile([128, 128], f32, tag="ident")
            wq_sb = cp.tile([128, KTILES, DPC], bf, tag="wq")
            wk_sb = cp.tile([128, KTILES, DPC], bf, tag="wk")
            wv_sb = cp.tile([128, KTILES, DPC], bf, tag="wv")

            warm = cp.tile([128, 128], bf, tag="warm")
            nc.vector.memset(warm[:], 0.5)
            wps = psP.tile([128, 128], f32, tag="P")
            for _ in range(10):
                nc.tensor.matmul(wps[:], warm[:], warm[:],
                                 start=True, stop=True)
            xt_tiles = {}

            def load_chunk(c):
                xt_t = p1.tile([128, KTILES, CHUNK], bf, tag="xt")
                half = KTILES // 2
                nc.sync.dma_start(out=xt_t[:, 0:half, :],
                                  in_=xt_d[c, :, 0:half, :])
                nc.sync.dma_start(out=xt_t[:, half:KTILES, :],
                                  in_=xt_d[c, :, half:KTILES, :])
                xt_tiles[c] = xt_t

            # DMA issue order = first-use order.  Keepalive matmul bursts
            # gated on arriving DMAs hold the PE p-state through the
            # input-stream window.
            nc.sync.dma_start(out=wq_sb[:], in_=wq_d[:])
            for k in range(6):
                nc.tensor.matmul(wps[:], wq_sb[:, k, :],
                                 wq_sb[:, k + 1, :], start=True, stop=True)
            load_chunk(0)
            for w_sb, w_d in ((wk_sb, wk_d), (wv_sb, wv_d)):
                nc.sync.dma_start(out=w_sb[:], in_=w_d[:])
            for k in range(6):
                nc.tensor.matmul(wps[:], wk_sb[:, k, :],
                                 wk_sb[:, k + 1, :], start=True, stop=True)
            nc.sync.dma_start(out=cos_sb[:], in_=cos_d[:])
            nc.sync.dma_start(out=sin_sb[:], in_=sin_d[:])
            load_chunk(1)
            nc.sync.dma_start(out=ident[:], in_=ident_d[:])
            for k in range(6):
                nc.tensor.matmul(wps[:], wv_sb[:, k, :],
                                 wv_sb[:, k + 1, :], start=True, stop=True)
            ones_sb = cp.tile([128, T // 128], bf, tag="ones")
            nc.vector.memset(ones_sb[:], 1.0)
            for h in range(HPC):
                nc.vector.tensor_copy(
                    vaug[:, :, h * VW + HD:h * VW + HD + 1], ones_sb[:])

            def chunk_pieces(c):
                """QKV projection + RoPE + V transpose for token chunk c,
                as a generator of small emission pieces (PE filler)."""
                if c not in xt_tiles:
                    load_chunk(c)
                xt_t = xt_tiles.pop(c)
                pos = (c * CHUNK) % S
                cs = cos_sb[:, pos:pos + CHUNK]
                sn = sin_sb[:, pos:pos + CHUNK]

                for w_sb, dst in ((wq_sb, qt), (wk_sb, kt)):
                    ps = psP.tile([DPC, CHUNK], f32, tag="P")
                    for k0 in range(0, KTILES, 4):
                        for k in range(k0, k0 + 4):
                            nc.tensor.matmul(
                                ps[:], w_sb[:, k, :], xt_t[:, k, :],
                                start=(k == 0), stop=(k == KTILES - 1))
                        yield
                    raw = p1s.tile([DPC, CHUNK], bf, tag="raw")
                    nc.vector.tensor_copy(raw[:], ps[:])
                    rot = p1s.tile([DPC, CHUNK], bf, tag="rot")
                    hh = HD // 2
                    for blk in range(DPC // hh):
                        sb = ((blk // 2) * 2) + (1 - blk % 2)
                        nc.sync.dma_start(
                            out=rot[blk * hh:(blk + 1) * hh, :],
                            in_=raw[sb * hh:(sb + 1) * hh, :])
                    t1 = p1s.tile([DPC, CHUNK], bf, tag="t1")
                    nc.vector.tensor_mul(t1[:], raw[:], cs)
                    t2 = p1s.tile([DPC, CHUNK], bf, tag="t2")
                    nc.vector.tensor_mul(t2[:], rot[:], sn)
                    nc.vector.tensor_add(
                        dst[:, c * CHUNK:(c + 1) * CHUNK], t1[:], t2[:])
                    yield

                psv = psP.tile([DPC, CHUNK], f32, tag="P")
                for k0 in range(0, KTILES, 4):
                    for k in range(k0, k0 + 4):
                        nc.tensor.matmul(
                            psv[:], wv_sb[:, k, :], xt_t[:, k, :],
                            start=(k == 0), stop=(k == KTILES - 1))
                    yield
                vt = p1s.tile([DPC, CHUNK], f32, tag="vt")
                nc.vector.tensor_copy(vt[:], psv[:])
                yield
                for j in range(CHUNK // 128):
                    tp = psP.tile([128, 128], f32, tag="P")
                    nc.tensor.transpose(tp[:], vt[:, j * 128:(j + 1) * 128],
                                        ident[:])
                    tt = c * (CHUNK // 128) + j
                    for h in range(HPC):
                        nc.vector.tensor_copy(
                            vaug[:, tt, h * VW:h * VW + HD],
                            tp[:, h * HD:(h + 1) * HD])
                    if j % 2 == 1:
                        yield

            def emit_chunk(c):
                for _ in chunk_pieces(c):
                    pass

            # ---------- attention pipeline ----------
            def emit_scores(b, h, qh, kc):
                hs = slice(h * HD, (h + 1) * HD)
                q0 = b * S + qh * QG
                k0 = b * S + kc * KC
                scp = psA.tile([128, QG], f32, tag="A")
                for qn in range(QG // 512):
                    nc.tensor.matmul(
                        scp[:, qn * 512:(qn + 1) * 512],
                        kt[hs, k0:k0 + KC],
                        qt[hs, q0 + qn * 512:q0 + (qn + 1) * 512],
                        start=True, stop=True)
                e = p2e.tile([128, QG], bf, tag="e")
                nc.scalar.activation(e[:], scp[:], Exp, scale=0.125)
                return e

            cx_state = {}

            def emit_ctx(b, h, qh, kc, e):
                if kc == 0:
                    cx_state["cx"] = psCX.tile([VW, QG], f32, tag="cx",
                                               name="cx")
                cx = cx_state["cx"]
                tt = (b * S) // 128 + kc
                for qn in range(QG // 512):
                    nc.tensor.matmul(
                        cx[:, qn * 512:(qn + 1) * 512],
                        vaug[:, tt, h * VW:(h + 1) * VW],
                        e[:, qn * 512:(qn + 1) * 512],
                        start=(kc == 0), stop=(kc == NKC - 1))
                if kc == NKC - 1:
                    q0 = b * S + qh * QG
                    ctxu = p2o.tile([VW, QG], f32, tag="ctxu")
                    nc.vector.tensor_copy(ctxu[:], cx[:])
                    rsqi = p2o.tile([1, QG], f32, tag="rsqi")
                    nc.vector.reciprocal(rsqi[:], ctxu[HD:VW, :])
                    rb = p2o.tile([HD, QG], f32, tag="rb")
                    nc.gpsimd.partition_broadcast(rb[:], rsqi[:])
                    oc = p2o.tile([HD, QG], f32, tag="oc")
                    nc.vector.tensor_mul(oc[:], ctxu[0:HD, :], rb[:])
                    nc.sync.dma_start(
                        out=ctxt_d[h * HD:(h + 1) * HD, q0:q0 + QG],
                        in_=oc[:])

            # chunks 0 and 1 fully up front (batch-0 attention needs them)
            emit_chunk(0)
            emit_chunk(1)

            # remaining phase-1 chunks woven into the attention stream
            import collections
            gens = collections.deque(chunk_pieces(c) for c in range(2, 8))
            pumped = [0]

            def pump_to(target):
                while pumped[0] < target and gens:
                    try:
                        next(gens[0])
                        pumped[0] += 1
                    except StopIteration:
                        gens.popleft()

            # piece targets: c2+c3 (22 pieces) by iteration 14, the
            # rest (44) spread over iterations 15..70
            def target(i):
                if i < 14:
                    return (22 * (i + 1) + 13) // 14
                return min(66, 22 + (44 * (i - 13) + 55) // 56)

            iters = []
            for b in (0, 1):
                for h, qh in ((0, 0), (1, 0), (0, 1), (1, 1)):
                    for kc in range(NKC):
                        iters.append((b, h, qh, kc))

            pending = collections.deque()
            for i, (b, h, qh, kc) in enumerate(iters):
                e = emit_scores(b, h, qh, kc)
                pending.append((b, h, qh, kc, e))
                pump_to(target(i))
                if len(pending) > LAG:
                    emit_ctx(*pending.popleft())
            while gens:
                pump_to(pumped[0] + 1)
            while pending:
                emit_ctx(*pending.popleft())

    nc.compile()
    return nc


def _host_tables():
    inv_freq = 1.0 / (10000.0 ** (np.arange(0, HD, 2, dtype=np.float32) / HD))
    t = np.arange(S, dtype=np.float32)
    freqs = np.outer(t, inv_freq)            # [S, 32]
    emb = np.concatenate([freqs, freqs], axis=-1)  # [S, 64]
    cosT = np.cos(emb).T.astype(np.float32)  # [64, S]
    sinT = np.sin(emb).T.astype(np.float32)
    sin_signed = sinT.copy()
    sin_signed[:HD // 2] *= -1.0             # rows d<32 multiply -sin
    cos2 = np.ascontiguousarray(np.vstack([cosT, cosT])).astype(BF16)
    sins = np.ascontiguousarray(np.vstack([sin_signed, sin_signed])).astype(BF16)
    ident = np.eye(128, dtype=np.float32)
    return cos2, sins, ident


def kernel(hidden_states: np.ndarray, Wq: np.ndarray, Wk: np.ndarray,
           Wv: np.ndarray) -> np.ndarray:
    hidden_states = np.asarray(hidden_states, dtype=np.float32)
    Wq = np.asarray(Wq, dtype=np.float32)
    Wk = np.asarray(Wk, dtype=np.float32)
    Wv = np.asarray(Wv, dtype=np.float32)
    assert hidden_states.shape == (B, S, H), hidden_states.shape

    if "nc" not in _prog_cache:
        _prog_cache["nc"] = _build_program()
    nc = _prog_cache["nc"]

    xt = hidden_states.reshape(T, H).T  # [1024, 4096] view
    # pre-tile so each chunk DMA is contiguous per partition:
    # xt_tiled[c, p, k, t] = xt[k*128 + p, c*512 + t]
    xt_tiled = np.ascontiguousarray(
        xt.reshape(KTILES, 128, NCHUNK, CHUNK).transpose(2, 1, 0, 3)
    ).astype(BF16)
    cos2, sins, ident = _host_tables()

    def tile_w(W):
        # w_tiled[p, k, d] = W.T[k*128 + p, d]
        return np.ascontiguousarray(
            W.T.reshape(KTILES, 128, DPC).transpose(1, 0, 2)).astype(BF16)

    in_maps = []
    for i in range(NCORES):
        rows = slice(i * DPC, (i + 1) * DPC)
        in_maps.append({
            "xt": xt_tiled,
            "wq": tile_w(Wq[rows]),
            "wk": tile_w(Wk[rows]),
            "wv": tile_w(Wv[rows]),
            "cos2": cos2,
            "sins": sins,
            "ident": ident,
        })

    global _last_in_maps
    _last_in_maps = in_maps
    res = run_bass_kernel_spmd(nc, in_maps, list(range(NCORES)))

    # ctxt per core: [128 (2 heads x 64 dims), 4096 (2 batches x 2048)]
    full = np.stack([res.results[i]["ctxt"] for i in range(NCORES)])
    out = full.reshape(NCORES, HPC, HD, B, S).transpose(3, 4, 0, 1, 2)
    return np.ascontiguousarray(out.reshape(B, S, H), dtype=np.float32)


# revision 17
# speedup vs baseline: 1.1706x; 1.0992x over previous
"""Multi-head attention (16 heads, head_dim 64, B=2, S=2048) on 8 trn2 cores.

Sharding: tensor-parallel over heads — core i computes heads 2i, 2i+1 for both
batch elements. Each core receives the full X^T [1024, 4096] in bf16 plus its
128-row slice of Wq/Wk/Wv (bf16, transposed to [1024, 128] lhsT layout), and
returns ctx^T [128, 4096] f32, already softmax-normalized.

v3 design (exp-stream-bound):
  All matmul operands bf16 (1 cycle/row, cheap LDWEIGHTS, half DMA), psum f32.
  Phase 1: QT/KT [128, 4096] bf16 = W^T.T @ X^T (8 k-tiles), RoPE on DVE
    (rotate-half via SBUF-SBUF DMA partition swap, signed-sin tables).
    V transposed token-major into vaug [128, 32, 2*65] bf16 with a ones
    column per head (softmax denominators fall out of the ctx matmul).
  Attention: one global pipeline over 128 (group, k-chunk) iterations,
    groups strictly sequential: per iteration
      scoresT [128k, 1024q] psum = KT_chunk.T @ QT   (1 matmul, psA x2)
      e bf16 = exp(scores/8)                          (ACT, the bottleneck)
      ctx[65, 1024] += Vaug_chunk.T @ e               (lagged 5 iterations)
    The ctx lag + 7 e-buffers keep the ACT stream dense across group
    boundaries while the normalize chain holds the ctx psum.
  Normalize: denom row 64 -> DMA-reshape [128,8] -> DVE reciprocal ->
    DMA back -> GpSimd partition-broadcast -> DVE multiply -> DMA out.
  Phase-1 work beyond chunks 0/1-q,k is emitted in <=2-matmul pieces,
  pumped by a per-piece due-iteration schedule to fill PE/ACT slack.
  Warm matmul bursts into the (idle) psA pool hold the PE p-state up
  through the DMA-bound head.
"""
import collections
import os
import sys

for _p in ("/opt/trn_rl_repo", "/root/.axon_site/_ro/trn_rl_repo"):
    if os.path.isdir(_p) and _p not in sys.path:
        sys.path.insert(0, _p)

import numpy as np
import ml_dtypes

import concourse.bass as bass  # noqa: F401
import concourse.mybir as mybir
import concourse.tile as tile
from concourse import bacc
from concourse.bass_utils import run_bass_kernel_spmd

dt = mybir.dt
BF16 = ml_dtypes.bfloat16

B, S, NH, HD = 2, 2048, 16, 64
H = NH * HD            # 1024
T = B * S              # 4096
NCORES = 8
HPC = NH // NCORES     # heads per core = 2
DPC = HPC * HD         # dims per core = 128
CHUNK = 512            # token chunk for QKV projection
NCHUNK = 8
KC = 128               # k-token chunk in attention
NKC = S // KC          # 16 per batch
QG = 1024              # q extent per attention group
VW = HD + 1            # 65: 64 dims then the ones column (softmax denom)
KTILES = H // 128      # 8
LAG = 5                # ctx matmul lag (iterations) behind scores/exp

_prog_cache = {}
_last_in_maps = None


def _build_program():
    nc = bacc.Bacc("TRN2", target_bir_lowering=False, debug=False,
                   num_devices=NCORES)
    f32 = dt.float32
    bf = dt.bfloat16

    xt_d = nc.declare_dram_parameter("xt", [NCHUNK, 128, KTILES, CHUNK], bf,
                                     isOutput=False)
    wq_d = nc.declare_dram_parameter("wq", [128, KTILES, DPC], bf,
                                     isOutput=False)
    wk_d = nc.declare_dram_parameter("wk", [128, KTILES, DPC], bf,
                                     isOutput=False)
    wv_d = nc.declare_dram_parameter("wv", [128, KTILES, DPC], bf,
                                     isOutput=False)
    cos_d = nc.declare_dram_parameter("cos2", [DPC, S], bf, isOutput=False)
    sin_d = nc.declare_dram_parameter("sins", [DPC, S], bf, isOutput=False)
    ident_d = nc.declare_dram_parameter("ident", [128, 128], f32,
                                        isOutput=False)
    ctxt_d = nc.declare_dram_parameter("ctxt", [DPC, T], f32, isOutput=True)

    Exp = mybir.ActivationFunctionType.Exp

    with tile.TileContext(nc) as tc:
        with (
            tc.tile_pool(name="persist", bufs=1) as pp,
            tc.tile_pool(name="consts", bufs=1) as cp,
            tc.tile_pool(name="p1", bufs=5) as p1,
            tc.tile_pool(name="p1s", bufs=3) as p1s,
            tc.tile_pool(name="p2e", bufs=8) as p2e,
            tc.tile_pool(name="p2o", bufs=2) as p2o,
            # PSUM: scores 2x2 banks + qk-proj 1 + v-proj/transpose 1
            #       + ctx 1x2 = 8
            tc.tile_pool(name="psA", bufs=2, space="PSUM") as psA,
            tc.tile_pool(name="psPq", bufs=1, space="PSUM") as psPq,
            tc.tile_pool(name="psPv", bufs=1, space="PSUM") as psPv,
            tc.tile_pool(name="psCX", bufs=1, space="PSUM") as psCX,
        ):
            qt = pp.tile([DPC, T], bf, tag="qt")
            kt = pp.tile([DPC, T], bf, tag="kt")
            vaug = pp.tile([128, T // 128, HPC * VW], bf, tag="vaug")

            cos_sb = cp.tile([DPC, S], bf, tag="cos")
            sin_sb = cp.tile([DPC, S], bf, tag="sin")
            ident = cp.tile([128, 128], f32, tag="ident")
            wq_sb = cp.tile([128, KTILES, DPC], bf, tag="wq")
            wk_sb = cp.tile([128, KTILES, DPC], bf, tag="wk")
            wv_sb = cp.tile([128, KTILES, DPC], bf, tag="wv")

            warm = cp.tile([128, 512], bf, tag="warm")
            nc.vector.memset(warm[:], 0.25)

            def warm_burst(n=2):
                wps = psA.tile([128, QG], f32, tag="A", name="wps")
                for i in range(n):
                    nc.tensor.matmul(wps[:, 0:512], warm[:, 0:128],
                                     warm[:], start=True, stop=True)

            xt_tiles = {}

            def load_chunk(c):
                if c in xt_tiles or c >= NCHUNK:
                    return
                xt_t = p1.tile([128, KTILES, CHUNK], bf, tag="xt")
                half = KTILES // 2
                nc.sync.dma_start(out=xt_t[:, 0:half, :],
                                  in_=xt_d[c, :, 0:half, :])
                nc.sync.dma_start(out=xt_t[:, half:KTILES, :],
                                  in_=xt_d[c, :, half:KTILES, :])
                xt_tiles[c] = xt_t

            # DMA issue order = first-use order; warm bursts into the idle
            # psA pool keep the PE p-state up through the DMA-bound head.
            warm_burst(4)
            nc.sync.dma_start(out=wq_sb[:], in_=wq_d[:])
            nc.sync.dma_start(out=cos_sb[:], in_=cos_d[:])
            warm_burst(2)
            nc.sync.dma_start(out=sin_sb[:], in_=sin_d[:])
            load_chunk(0)
            warm_burst(2)
            nc.sync.dma_start(out=wk_sb[:], in_=wk_d[:])
            load_chunk(1)
            warm_burst(2)
            nc.sync.dma_start(out=wv_sb[:], in_=wv_d[:])
            nc.sync.dma_start(out=ident[:], in_=ident_d[:])

            ones_sb = cp.tile([128, T // 128], bf, tag="ones")
            nc.vector.memset(ones_sb[:], 1.0)
            for h in range(HPC):
                nc.vector.tensor_copy(
                    vaug[:, :, h * VW + HD:h * VW + HD + 1], ones_sb[:])

            def qk_pieces(c, warm_fill=False):
                """QKV projection + RoPE for q,k of chunk c (2-MM pieces)."""
                load_chunk(c)
                xt_t = xt_tiles[c]
                pos = (c * CHUNK) % S
                cs = cos_sb[:, pos:pos + CHUNK]
                sn = sin_sb[:, pos:pos + CHUNK]
                for w_sb, dst in ((wq_sb, qt), (wk_sb, kt)):
                    ps = psPq.tile([DPC, CHUNK], f32, tag="Pq")
                    for k0 in range(0, KTILES, 2):
                        for k in range(k0, k0 + 2):
                            nc.tensor.matmul(
                                ps[:], w_sb[:, k, :], xt_t[:, k, :],
                                start=(k == 0), stop=(k == KTILES - 1))
                        if warm_fill:
                            warm_burst(1)
                        yield
                    raw = p1s.tile([DPC, CHUNK], bf, tag="raw")
                    nc.vector.tensor_copy(raw[:], ps[:])
                    t1 = p1s.tile([DPC, CHUNK], bf, tag="t1")
                    nc.vector.tensor_mul(t1[:], raw[:], cs)
                    rot = p1s.tile([DPC, CHUNK], bf, tag="rot")
                    hh = HD // 2
                    for blk in range(DPC // hh):
                        sb = ((blk // 2) * 2) + (1 - blk % 2)
                        nc.sync.dma_start(
                            out=rot[blk * hh:(blk + 1) * hh, :],
                            in_=raw[sb * hh:(sb + 1) * hh, :])
                    if warm_fill:
                        warm_burst(2)
                    yield
                    t2 = p1s.tile([DPC, CHUNK], bf, tag="t2")
                    nc.vector.tensor_mul(t2[:], rot[:], sn)
                    nc.vector.tensor_add(
                        dst[:, c * CHUNK:(c + 1) * CHUNK], t1[:], t2[:])
                    yield

            def v_pieces(c):
                """V projection + transpose into vaug for chunk c."""
                xt_t = xt_tiles[c]
                psv = psPv.tile([DPC, CHUNK], f32, tag="Pv")
                for k0 in range(0, KTILES, 2):
                    for k in range(k0, k0 + 2):
                        nc.tensor.matmul(
                            psv[:], wv_sb[:, k, :], xt_t[:, k, :],
                            start=(k == 0), stop=(k == KTILES - 1))
                    yield
                vt = p1s.tile([DPC, CHUNK], f32, tag="vt")
                nc.vector.tensor_copy(vt[:], psv[:])
                yield
                for j in range(CHUNK // 128):
                    tp = psPv.tile([128, 128], f32, tag="Pv")
                    nc.tensor.transpose(tp[:], vt[:, j * 128:(j + 1) * 128],
                                        ident[:])
                    tt = c * (CHUNK // 128) + j
                    for h in range(HPC):
                        nc.vector.tensor_copy(
                            vaug[:, tt, h * VW:h * VW + HD],
                            tp[:, h * HD:(h + 1) * HD])
                    yield

            def chunk_done(c):
                xt_tiles.pop(c, None)

            # ---------- attention pipeline ----------
            def emit_scores(b, h, qh, kc):
                hs = slice(h * HD, (h + 1) * HD)
                q0 = b * S + qh * QG
                k0 = b * S + kc * KC
                scp = psA.tile([128, QG], f32, tag="A")
                for qn in range(QG // 512):
                    nc.tensor.matmul(
                        scp[:, qn * 512:(qn + 1) * 512],
                        kt[hs, k0:k0 + KC],
                        qt[hs, q0 + qn * 512:q0 + (qn + 1) * 512],
                        start=True, stop=True)
                e = p2e.tile([128, QG], bf, tag="e")
                nc.scalar.activation(e[:], scp[:], Exp, scale=0.125)
                return e

            cx_state = {}

            def emit_ctx(b, h, qh, kc, e):
                if kc == 0:
                    cx_state["cx"] = psCX.tile([VW, QG], f32, tag="cx",
                                               name="cx")
                cx = cx_state["cx"]
                tt = (b * S) // 128 + kc
                for qn in range(QG // 512):
                    nc.tensor.matmul(
                        cx[:, qn * 512:(qn + 1) * 512],
                        vaug[:, tt, h * VW:(h + 1) * VW],
                        e[:, qn * 512:(qn + 1) * 512],
                        start=(kc == 0), stop=(kc == NKC - 1))
                if kc == NKC - 1:
                    q0 = b * S + qh * QG
                    ctxu = p2o.tile([VW, QG], f32, tag="ctxu")
                    nc.vector.tensor_copy(ctxu[:], cx[:])
                    rsq = p2o.tile([128, QG // 128], f32, tag="rsq")
                    nc.sync.dma_start(out=rsq[:], in_=ctxu[HD:VW, :])
                    rsqi = p2o.tile([128, QG // 128], f32, tag="rsqi")
                    nc.vector.reciprocal(rsqi[:], rsq[:])
                    r0 = p2o.tile([1, QG], f32, tag="r0")
                    nc.sync.dma_start(out=r0[:], in_=rsqi[:])
                    rb = p2o.tile([HD, QG], f32, tag="rb")
                    nc.gpsimd.partition_broadcast(rb[:], r0[:])
                    oc = p2o.tile([HD, QG], f32, tag="oc")
                    nc.vector.tensor_mul(oc[:], ctxu[0:HD, :], rb[:])
                    nc.sync.dma_start(
                        out=ctxt_d[h * HD:(h + 1) * HD, q0:q0 + QG],
                        in_=oc[:])

            # head: q,k of chunks 0/1 (with warm fill); V is woven later
            for _ in qk_pieces(0, warm_fill=True):
                pass
            for _ in qk_pieces(1, warm_fill=True):
                pass
            load_chunk(2)
            load_chunk(3)
            load_chunk(4)

            # phase-1 weave: per-generator linear due-iteration schedules
            weave = collections.deque()

            def add(due_lo, due_hi, gen, n_hint):
                # spread dues linearly over [due_lo, due_hi]
                weave.append([due_lo, due_hi, gen, n_hint, 0])

            # two lanes with sequential windows each: qk-lane on psPq,
            # v-lane on psPv — no shared psum ring between concurrent
            # generators (deadlock safety), deadlines with >=1 iter slack
            add(0, 5, qk_pieces(2), 12)    # kt c2 needed at iter 8
            add(6, 11, qk_pieces(3), 12)   # kt c3 needed at iter 12
            add(14, 40, qk_pieces(4), 12)  # q/k c4 needed at iter 64
            add(40, 54, qk_pieces(5), 12)  # q/k c5 needed at iter 64
            add(54, 66, qk_pieces(6), 12)  # kt c6 needed at iter 72
            add(66, 74, qk_pieces(7), 12)  # kt c7 needed at iter 76
            add(0, 4, v_pieces(0), 9)      # vaug c0 needed at iter 5
            add(4, 8, v_pieces(1), 9)      # vaug c1 needed at iter 9
            add(9, 12, v_pieces(2), 9)     # vaug c2 needed at iter 13
            add(13, 16, v_pieces(3), 9)    # vaug c3 needed at iter 17
            add(20, 66, v_pieces(4), 9)    # vaug c4 needed at iter 69
            add(66, 72, v_pieces(5), 9)    # vaug c5 needed at iter 73
            add(72, 76, v_pieces(6), 9)    # vaug c6 needed at iter 77
            add(76, 80, v_pieces(7), 9)    # vaug c7 needed at iter 81

            def pump(i):
                # advance every weave entry whose linear schedule is due
                for entry in list(weave):
                    lo, hi, gen, n, done = entry
                    while True:
                        due = lo + (hi - lo) * entry[4] / max(n, 1)
                        if due > i:
                            break
                        try:
                            next(gen)
                            entry[4] += 1
                        except StopIteration:
                            weave.remove(entry)
                            break

            iters = []
            for b in (0, 1):
                for h, qh in ((0, 0), (1, 0), (0, 1), (1, 1)):
                    for kc in range(NKC):
                        iters.append((b, h, qh, kc))

            pending = collections.deque()
            for i, (b, h, qh, kc) in enumerate(iters):
                pump(i)
                e = emit_scores(b, h, qh, kc)
                pending.append((b, h, qh, kc, e))
                if i == 30:
                    load_chunk(5)
                if i == 45:
                    load_chunk(6)
                if i == 58:
                    load_chunk(7)
                if len(pending) > LAG:
                    emit_ctx(*pending.popleft())
                # drain the lag early near the end so the tail is short
                if i >= len(iters) - LAG and pending:
                    emit_ctx(*pending.popleft())
            while weave:
                lo, hi, gen, n, done = weave[0]
                try:
                    next(gen)
                except StopIteration:
                    weave.popleft()
            while pending:
                emit_ctx(*pending.popleft())

    nc.compile()
    return nc


def _host_tables():
    inv_freq = 1.0 / (10000.0 ** (np.arange(0, HD, 2, dtype=np.float32) / HD))
    t = np.arange(S, dtype=np.float32)
    freqs = np.outer(t, inv_freq)            # [S, 32]
    emb = np.concatenate([freqs, freqs], axis=-1)  # [S, 64]
    cosT = np.cos(emb).T.astype(np.float32)  # [64, S]
    sinT = np.sin(emb).T.astype(np.float32)
    sin_signed = sinT.copy()
    sin_signed[:HD // 2] *= -1.0             # rows d<32 multiply -sin
    cos2 = np.ascontiguousarray(np.vstack([cosT, cosT])).astype(BF16)
    sins = np.ascontiguousarray(np.vstack([sin_signed, sin_signed])).astype(BF16)
    ident = np.eye(128, dtype=np.float32)
    return cos2, sins, ident


def kernel(hidden_states: np.ndarray, Wq: np.ndarray, Wk: np.ndarray,
           Wv: np.ndarray) -> np.ndarray:
    hidden_states = np.asarray(hidden_states, dtype=np.float32)
    Wq = np.asarray(Wq, dtype=np.float32)
    Wk = np.asarray(Wk, dtype=np.float32)
    Wv = np.asarray(Wv, dtype=np.float32)
    assert hidden_states.shape == (B, S, H), hidden_states.shape

    if "nc" not in _prog_cache:
        _prog_cache["nc"] = _build_program()
    nc = _prog_cache["nc"]

    xt = hidden_states.reshape(T, H).T  # [1024, 4096] view
    # pre-tile so each chunk DMA is contiguous per partition:
    # xt_tiled[c, p, k, t] = xt[k*128 + p, c*512 + t]
    xt_tiled = np.ascontiguousarray(
        xt.reshape(KTILES, 128, NCHUNK, CHUNK).transpose(2, 1, 0, 3)
    ).astype(BF16)
    cos2, sins, ident = _host_tables()

    def tile_w(W):
        # w_tiled[p, k, d] = W.T[k*128 + p, d]
        return np.ascontiguousarray(
            W.T.reshape(KTILES, 128, DPC).transpose(1, 0, 2)).astype(BF16)

    in_maps = []
    for i in range(NCORES):
        rows = slice(i * DPC, (i + 1) * DPC)
        in_maps.append({
            "xt": xt_tiled,
            "wq": tile_w(Wq[rows]),
            "wk": tile_w(Wk[rows]),
            "wv": tile_w(Wv[rows]),
            "cos2": cos2,
            "sins": sins,
            "ident": ident,
        })

    global _last_in_maps
    _last_in_maps = in_maps
    res = run_bass_kernel_spmd(nc, in_maps, list(range(NCORES)))

    # ctxt per core: [128 (2 heads x 64 dims), 4096 (2 batches x 2048)]
    full = np.stack([res.results[i]["ctxt"] for i in range(NCORES)])
    out = full.reshape(NCORES, HPC, HD, B, S).transpose(3, 4, 0, 1, 2)
    return np.ascontiguousarray(out.reshape(B, S, H), dtype=np.float32)
